# revision 96
# baseline (speedup 1.0000x reference)
"""Trainium2 Bass kernel for pre-norm causal attention block.

Module: out = x + Wo(attn(LN(x))) with fused QKV, 16 heads, causal mask.
Shapes (hardcoded): x [2, 2048, 1024], wqkv [1024, 3072], wo [1024, 1024].

Sharding (8 cores, one program SPMD):
  core c = 4*b + s handles batch b, global heads [4s, 4s+4). Attention
  context is normalized on the sender, then exchanged with 4 chunked
  AllToAlls (one per 512-wide q block, each shipping a 64-col slice per
  destination) so the exchange overlaps attention compute. Core c's output
  rows are seq positions {qt*512 + 64*c + q : qt in 0..3, q in 0..63} of
  both batches.

Dataflow per core (feature-on-partitions, i.e. transposed):
  1. x, wqkv, wv arrive as fp8-e4m3 in DoubleRow-packed layout (weights
     pre-scaled x64 on the host to dodge e4m3 subnormals). LN stats via
     DoubleRow ones-matmuls on PE; rsqrt via fast-inverse-sqrt + Newton on
     DVE. LN is folded into the projections via two extra bf16 contraction
     rows: psum = 64*Wg^T x^T - 64*C*mu + 64*b2*(1/r); qkv^T = (r/64)*psum,
     so the per-tile epilogue is one DVE scalar_tensor_tensor with a
     0-stride-DMA broadcast of r.
  2. QK projection (fp8 DoubleRow, K=256/step) -> qk^T bf16 [512, 2048];
     V projection with swapped operands -> V row layout [2048, 256] (+ a
     ones column per head for softmax denominators), epilogue = one
     tensor_scalar by r/64 (per-partition).
  3. Per head: scores^T blocks [128 k, q] (bf16, K=64) for the visible
     (causal) q-range only, exp on ACT (scale=1/8 folded in), 128x128
     triangular mask on the diagonal block, ctx^T = V_aug^T @ expS (bf16)
     accumulated in PSUM [65, 512] (row 64 = softmax denominator).
  4. Per qt: ctx copied out of PSUM fast (frees banks), denominator
     reciprocals via Newton on a partition-{0,32,64,96} strided tile,
     broadcast via one selector matmul per head group, normalize into ctxT
     (bf16), ship per-dest 64-col slices with merged DMAs, then one
     AllToAll per qt so chunks 0-2 overlap attention compute (a tiny
     warmup AllToAll at kernel start absorbs the CC ring-setup cost).
  5. Receiver: assemble ctx_all [128, 512] (cols ordered chunk-major),
     output projection split into pass A (chunks 0-2, overlaps the last
     AllToAll) and pass B + residual (bias pre-folded into xres on host).
"""

import sys

for _p in ("/opt/trn_rl_repo",):
    if _p not in sys.path:
        sys.path.insert(0, _p)

import math

import ml_dtypes
import numpy as np

import concourse.bass as bass
import concourse.mybir as mybir
import concourse.tile as tile
from concourse import bacc
from concourse.bass_utils import run_bass_kernel_spmd

F32 = mybir.dt.float32
BF16 = mybir.dt.bfloat16
FP8 = mybir.dt.float8e4
I32 = mybir.dt.int32
AF = mybir.ActivationFunctionType
ALU = mybir.AluOpType
DR = mybir.MatmulPerfMode.DoubleRow
WS = 64.0                # fp8 weight scale (avoids e4m3 subnormals)

N_CORES = 8
B, S, H, D = 2, 2048, 16, 64
DIM = H * D              # 1024
HL = 4                   # heads per core
DL = HL * D              # 256 local head features
EPS = 1e-6
KT = 128                 # k-tile (partition) width
NT = 512                 # matmul free-dim tile
FT = DIM // KT           # 8 feature tiles
ST = S // KT             # 16 seq tiles of 128
QT = S // NT             # 4 q-tiles of 512
CH = 64                  # per-destination column slice per chunk

_CACHE = {}


def _build():
    nc = bacc.Bacc("TRN2", target_bir_lowering=False, debug=False,
                   num_devices=N_CORES)

    # ---- I/O ----
    # DoubleRow-packed fp8: row j*128+p holds features (256j+p, 256j+128+p)
    # interleaved as [2, N] blocks
    xdr_d = nc.dram_tensor("xdr", [DIM // 2, 2 * S], FP8, kind="ExternalInput")
    xres_d = nc.dram_tensor("xres", [DIM, NT], F32, kind="ExternalInput")
    wqk_d = nc.dram_tensor("wqk", [DIM // 2, 4 * DL], FP8,
                           kind="ExternalInput")
    wv_d = nc.dram_tensor("wv", [DIM // 2, 2 * DL], FP8, kind="ExternalInput")
    wo_d = nc.dram_tensor("wo", [DIM, DIM], BF16, kind="ExternalInput")
    qcor_d = nc.dram_tensor("qcor", [2, 2 * DL], BF16, kind="ExternalInput")
    vcor_d = nc.dram_tensor("vcor", [2, DL], BF16, kind="ExternalInput")
    tri_d = nc.dram_tensor("tri", [128, 128], BF16, kind="ExternalInput")
    y_d = nc.dram_tensor("y", [DIM, NT], F32, kind="ExternalOutput")

    # ---- DRAM scratch ----
    stats_dram = nc.dram_tensor("stats_dram", [2, S], F32)
    rec_dram = nc.dram_tensor("rec_dram", [QT, HL, NT], F32)
    recip_dram = nc.dram_tensor("recip_dram", [QT, HL, NT], F32)
    r_dram = nc.dram_tensor("r_dram", [S], F32)
    a2a_in = [nc.dram_tensor(f"a2a_in{k}", [N_CORES, 2 * KT, CH], BF16)
              for k in range(QT)]
    a2a_out = [nc.dram_tensor(f"a2a_out{k}", [N_CORES, 2 * KT, CH], BF16)
               for k in range(QT)]
    warm_in = nc.dram_tensor("warm_in", [N_CORES, 1, CH], BF16)
    warm_out = nc.dram_tensor("warm_out", [N_CORES, 1, CH], BF16)

    with tile.TileContext(nc) as tc:
        import contextlib
        with contextlib.ExitStack() as ctx:
            _build_body(ctx, tc, nc, locals())
    nc.compile()
    return nc


def _build_body(ctx, tc, nc, t):
    xdr_d, xres_d, wqk_d, wv_d, wo_d = (t["xdr_d"], t["xres_d"], t["wqk_d"],
                                        t["wv_d"], t["wo_d"])
    qcor_d, vcor_d, tri_d, y_d = t["qcor_d"], t["vcor_d"], t["tri_d"], t["y_d"]
    stats_dram, a2a_in, a2a_out = t["stats_dram"], t["a2a_in"], t["a2a_out"]
    rec_dram = t["rec_dram"]
    recip_dram = t["recip_dram"]
    r_dram = t["r_dram"]

    P = 128
    sing = ctx.enter_context(tc.tile_pool(name="sing", bufs=1))
    # persistent SBUF tiles
    JT = FT // 2             # 4 DoubleRow feature steps of 256
    xdr = [sing.tile([P, 2, S], FP8, tag=f"xdr{j}", name=f"xdr{j}")
           for j in range(JT)]
    xres = [sing.tile([P, NT], F32, tag=f"xres{i}", name=f"xres{i}") for i in range(FT)]
    wqk = [sing.tile([P, 2, 2 * DL], FP8, tag=f"wqk{j}", name=f"wqk{j}")
           for j in range(JT)]
    wv = [sing.tile([P, 2, DL], FP8, tag=f"wv{j}", name=f"wv{j}")
          for j in range(JT)]
    wo = [sing.tile([P, DIM], BF16, tag=f"wo{i}", name=f"wo{i}")
          for i in range(FT)]
    qkT = [sing.tile([P, S], BF16, tag=f"qkT{i}", name=f"qkT{i}") for i in range(4)]
    vaug = [sing.tile([P, HL * (D + 1)], BF16, tag=f"vaug{i}", name=f"vaug{i}")
            for i in range(ST)]
    ctxT = [sing.tile([P, S], BF16, tag=f"ctxT{i}", name=f"ctxT{i}") for i in range(2)]
    # chunks 0-2 and chunk 3 in physically separate tiles so the pass-A
    # output projection has no (tile-granular) dependency on the last chunk
    ctxA = [sing.tile([P, 6 * CH], BF16, tag=f"cta{i}", name=f"cta{i}")
            for i in range(FT)]
    ctxB = [sing.tile([P, 2 * CH], BF16, tag=f"ctb{i}", name=f"ctb{i}")
            for i in range(FT)]
    rB = sing.tile([P, S], F32, tag="rB")
    mrow = sing.tile([2, S], BF16, tag="mrow")
    qcor_c = sing.tile([2, 2 * DL], BF16, tag="qcor")
    vcor_c = sing.tile([2, DL], BF16, tag="vcor")
    tri = sing.tile([P, P], BF16, tag="tri")
    ones = sing.tile([P, 1], BF16, tag="ones")

    rcP = sing.tile([P, ST], F32, tag="rcP")

    # input DMAs - x first (stats critical path), weights next, rest last
    for j in range(JT):
        nc.sync.dma_start(
            out=xdr[j],
            in_=xdr_d[j * P:(j + 1) * P, :].rearrange("p (t s) -> p t s", t=2))
    for j in range(JT):
        nc.sync.dma_start(
            out=wqk[j],
            in_=wqk_d[j * P:(j + 1) * P, :].rearrange("p (t s) -> p t s", t=2))
    for j in range(JT):
        nc.sync.dma_start(
            out=wv[j],
            in_=wv_d[j * P:(j + 1) * P, :].rearrange("p (t s) -> p t s", t=2))
    nc.sync.dma_start(out=qcor_c, in_=qcor_d[:])
    nc.sync.dma_start(out=vcor_c, in_=vcor_d[:])
    nc.sync.dma_start(out=tri, in_=tri_d[:])
    for i in range(FT):
        nc.sync.dma_start(out=wo[i], in_=wo_d[i * P:(i + 1) * P, :])
        nc.sync.dma_start(out=xres[i], in_=xres_d[i * P:(i + 1) * P, :])
    nc.vector.memset(ones, 1.0)

    # tiny warmup AllToAll: pays the CC ring-setup cost (~45us) during the
    # stats/QK phase so the real chunked exchanges run at steady state
    nc.gpsimd.collective_compute(
        "AllToAll", ALU.bypass,
        replica_groups=[list(range(N_CORES))],
        ins=[t["warm_in"][:].opt()], outs=[t["warm_out"][:].opt()],
        unique_tensors="Yes")

    # ---- 1. LN stats: column sums of x and x^2 via DoubleRow ones-matmul ----
    ones_f8 = sing.tile([P, 2, 16], FP8, tag="ones_f8")
    nc.vector.memset(ones_f8, 1.0)
    stats_sa = sing.tile([1, S], F32, tag="stats_sa")
    stats_sq = sing.tile([1, S], F32, tag="stats_sq")
    # stats use only 2 PSUM banks (sum/sq per nt sequentially) so the QK
    # pool below coexists and the PE never stalls on a pool release
    with tc.tile_pool(name="ps_st", bufs=4, space="PSUM") as ps_st, \
         tc.tile_pool(name="sqp", bufs=2) as sqp:
        sps = [ps_st.tile([1, NT], F32, tag="sum", name=f"sum{nt}")
               for nt in range(QT)]
        qps = [ps_st.tile([1, NT], F32, tag="sq", name=f"sqp{nt}")
               for nt in range(QT)]
        for j in range(JT):
            sq = sqp.tile([P, 2, S], FP8, tag="sq", name="sq")
            # squares on ACT (idle at startup; Square shares the exp table)
            # so the DVE queue stays clear for the stats->mrow chain
            nc.scalar.activation(sq, xdr[j], AF.Square)
            for nt in range(QT):
                sl = slice(nt * NT, (nt + 1) * NT)
                nc.tensor.matmul(sps[nt], ones_f8[:, :, 0:1],
                                 xdr[j][:, :, sl],
                                 start=(j == 0), stop=(j == JT - 1),
                                 perf_mode=DR)
                nc.tensor.matmul(qps[nt], ones_f8[:, :, 0:1], sq[:, :, sl],
                                 start=(j == 0), stop=(j == JT - 1),
                                 perf_mode=DR)
        for nt in range(QT):
            sl = slice(nt * NT, (nt + 1) * NT)
            nc.vector.tensor_copy(stats_sa[:, sl], sps[nt])
            nc.vector.tensor_copy(stats_sq[:, sl], qps[nt])
    nc.sync.dma_start(out=stats_dram[0:1], in_=stats_sa[:])
    nc.sync.dma_start(out=stats_dram[1:2], in_=stats_sq[:])
    # contiguous [16,128] reads, math at 16 partitions, then flatten
    # (SBUF->SBUF DMA) for the correction rows / broadcasts
    idn = sing.tile([P, P], F32, tag="idn")
    from concourse.masks import make_identity
    make_identity(nc, idn)
    sPT = sing.tile([16, P], F32, tag="sPT")
    qPT = sing.tile([16, P], F32, tag="qPT")
    nc.sync.dma_start(out=sPT, in_=stats_dram[0].rearrange("(j p) -> j p", j=16))
    nc.sync.dma_start(out=qPT, in_=stats_dram[1].rearrange("(j p) -> j p", j=16))
    muT = sing.tile([16, P], F32, tag="muT")
    nc.vector.tensor_scalar(muT, sPT, 1.0 / DIM, None, op0=ALU.mult)
    nc.vector.tensor_scalar(qPT, qPT, 1.0 / DIM, None, op0=ALU.mult)
    t0 = sing.tile([16, P], F32, tag="t0")
    nc.vector.tensor_mul(t0, muT, muT)
    nc.vector.tensor_sub(t0, qPT, t0)
    nc.vector.tensor_scalar(t0, t0, EPS, None, op0=ALU.add)
    # rsqrt via fast-inverse-square-root seed + 3 Newton steps
    rT = sing.tile([16, P], F32, tag="rT")
    t1s = sing.tile([16, P], F32, tag="t1s")
    nc.vector.tensor_scalar(rT[:].bitcast(I32), t0[:].bitcast(I32), 1, None,
                            op0=ALU.logical_shift_right)
    nc.vector.tensor_scalar(rT[:].bitcast(I32), rT[:].bitcast(I32), -1, None,
                            op0=ALU.bitwise_xor)
    nc.vector.tensor_scalar(rT[:].bitcast(I32), rT[:].bitcast(I32),
                            0x5F3759E0, None, op0=ALU.add)
    for _ in range(2):
        nc.vector.tensor_mul(t1s, rT, rT)
        nc.vector.tensor_mul(t1s, t1s, t0)
        nc.vector.tensor_scalar(t1s, t1s, -0.5, 1.5, op0=ALU.mult,
                                op1=ALU.add)
        nc.vector.tensor_mul(rT, rT, t1s)
    # invr = sqrt(var+eps) = t0 * r ; bf16 copies of mu and invr for the
    # correction rows
    invrT = sing.tile([16, P], F32, tag="invrT")
    nc.vector.tensor_mul(invrT, t0, rT)
    muB = sing.tile([16, P], BF16, tag="muB")
    invrB = sing.tile([16, P], BF16, tag="invrB")
    nc.vector.tensor_copy(muB, muT)
    nc.vector.tensor_copy(invrB, invrT)
    # mrow rows: [mu; invr] in seq order (partition-major flatten)
    nc.sync.dma_start(out=mrow[0:1, :], in_=muB[:])
    nc.sync.dma_start(out=mrow[1:2, :], in_=invrB[:])
    # r to DRAM, then 0-stride DMA broadcast to all 128 partitions
    nc.sync.dma_start(out=r_dram[:], in_=rT[:])
    nc.sync.dma_start(out=rB[:], in_=r_dram[:].partition_broadcast(P))

    # ---- 2. QK projection with LN correction rows ----
    with tc.tile_pool(name="ps_qk", bufs=8, space="PSUM") as ps_qk:
        for mt in range(4):          # qkT M-tiles (Q01 Q23 K01 K23)
            pss = []
            for nt in range(QT):
                sl = slice(nt * NT, (nt + 1) * NT)
                ps = ps_qk.tile([P, NT], F32, tag="qk", name=f"qk{nt}")
                for j in range(JT):
                    nc.tensor.matmul(
                        ps, wqk[j][:, :, mt * P:(mt + 1) * P],
                        xdr[j][:, :, sl],
                        start=(j == 0), stop=False, perf_mode=DR)
                pss.append(ps)
            for nt in range(QT):
                sl = slice(nt * NT, (nt + 1) * NT)
                nc.tensor.matmul(pss[nt], qcor_c[:, mt * P:(mt + 1) * P],
                                 mrow[:, sl], start=False, stop=True)
                nc.vector.scalar_tensor_tensor(
                    qkT[mt][:, sl], pss[nt], 1.0 / WS, rB[:, sl],
                    op0=ALU.mult, op1=ALU.mult)
    # column layout of r for the V epilogue via PE transpose (emitted after
    # the QK matmuls so the rT dependency never stalls the PE)
    with tc.tile_pool(name="ps_tp", bufs=1, space="PSUM") as ps_tp:
        tp = ps_tp.tile([P, 16], F32, tag="tp", name="tp")
        nc.tensor.transpose(tp, rT[:], idn[0:16, 0:16])
        # fold the 1/WS fp8 weight descale into the per-partition r column
        nc.vector.tensor_scalar(rcP, tp, 1.0 / WS, None, op0=ALU.mult)

    # selectors for the recip-broadcast matmuls (head hl's recip row lives
    # on partition 32*hl): selg[hg] maps rows {64hg, 64hg+32} to output
    # partition halves; zero rows null out the never-written garbage lanes
    selg = [sing.tile([97, P], BF16, tag=f"selg{hg}", name=f"selg{hg}")
            for hg in range(2)]
    for hg in range(2):
        nc.vector.memset(selg[hg], 0.0)
        nc.vector.memset(selg[hg][64 * hg:64 * hg + 1, 0:64], 1.0)
        nc.vector.memset(selg[hg][64 * hg + 32:64 * hg + 33, 64:128], 1.0)

    # ---- 3. attention: q-outer, 4 heads interleaved, V proj woven in ----
    with tc.tile_pool(name="ps_sc", bufs=2, space="PSUM") as ps_sc, \
         tc.tile_pool(name="ps_cx", bufs=1, space="PSUM") as ps_cx, \
         tc.tile_pool(name="ps_v", bufs=1, space="PSUM") as ps_v, \
         tc.tile_pool(name="ps_rb", bufs=1, space="PSUM") as ps_rb, \
         tc.tile_pool(name="esp", bufs=4) as esp, \
         tc.tile_pool(name="recp", bufs=2) as recp, \
         tc.tile_pool(name="rbp", bufs=4) as rbp:
        v_done = set()

        def weave_v(st):
            if st in v_done:
                return
            v_done.add(st)
            ps = ps_v.tile([P, DL], F32, tag="v", name="v")
            for j in range(JT):
                nc.tensor.matmul(
                    ps, xdr[j][:, :, st * P:(st + 1) * P], wv[j],
                    start=(j == 0), stop=False, perf_mode=DR)
            nc.tensor.matmul(ps, mrow[:, st * P:(st + 1) * P], vcor_c[:],
                             start=False, stop=True)
            vw = vaug[st][:].rearrange("p (h e) -> p h e", h=HL)
            nc.vector.tensor_scalar(
                vw[:, :, 0:D], ps.rearrange("p (h d) -> p h d", h=HL),
                rcP[:, st:st + 1], None, op0=ALU.mult)
            nc.vector.memset(vw[:, :, D:D + 1], 1.0)

        def finish_qt(qt, q0, den4, ctxR):
            """Newton reciprocal + normalize + ship for one q-block; emitted
            a few k-tiles into the NEXT q-block so the PE never head-of-line
            blocks on this (DVE-serial) chain at the boundary."""
            y0 = recp.tile([97, NT], F32, tag="y0", name="y0")
            a4 = recp.tile([97, NT], F32, tag="a4", name="a4")
            y0b = recp.tile([97, NT], BF16, tag="y0b", name="y0b")
            nc.vector.tensor_scalar(y0[:].bitcast(I32), den4[:].bitcast(I32),
                                    -1, None, op0=ALU.bitwise_xor)
            nc.vector.tensor_scalar(y0[:].bitcast(I32), y0[:].bitcast(I32),
                                    0x7EF311C4, None, op0=ALU.add)
            for _ in range(2):
                nc.vector.tensor_mul(a4, den4, y0)
                nc.vector.tensor_scalar(a4, a4, -1.0, 2.0,
                                        op0=ALU.mult, op1=ALU.add)
                nc.vector.tensor_mul(y0, y0, a4)
            nc.vector.tensor_copy(y0b, y0)
            # per head group: broadcast both recip rows via one selector
            # matmul, then normalize while writing into ctxT (bf16)
            for hg in range(2):
                rbps = ps_rb.tile([P, NT], F32, tag="rb", name="rb")
                nc.tensor.matmul(rbps, selg[hg][:], y0b[:],
                                 start=True, stop=True)
                for u in range(2):
                    hl = 2 * hg + u
                    nc.vector.tensor_mul(
                        ctxT[hg][64 * u:64 * u + 64, q0:q0 + NT],
                        ctxR[hl][0:D, :], rbps[64 * u:64 * u + 64, :])
            # ship this qt's 64-col window to each destination core (one
            # merged DMA per ctxT tile)
            for g in range(2):
                nc.sync.dma_start(
                    out=a2a_in[qt][:, P * g:P * (g + 1), :].rearrange(
                        "j r q -> r j q"),
                    in_=ctxT[g][:, q0:q0 + NT].rearrange(
                        "p (j q) -> p j q", j=N_CORES))
            nc.gpsimd.collective_compute(
                "AllToAll", ALU.bypass,
                replica_groups=[list(range(N_CORES))],
                ins=[a2a_in[qt][:].opt()], outs=[a2a_out[qt][:].opt()],
                unique_tensors="Yes")

        pending = None
        for qt in range(QT):
            q0 = qt * NT
            den4 = recp.tile([97, NT], F32, tag="den4", name="den4")
            nc.vector.memset(den4, 1.0)
            ctxR = [None] * HL
            # two head-group passes: only 2 ctx accumulators live at once
            # (2 PSUM banks) and each exp covers both heads of the group in
            # one [128, 2, 512] instruction (half the ACT dispatch overhead)
            for hg in range(2):
                cxs2 = [ps_cx.tile([D + 1, NT], F32, tag=f"cx{u}",
                                   name=f"cx{u}") for u in range(2)]
                for kt in range(4 * qt + 4):
                    k0 = kt * KT
                    weave_v(kt)
                    if pending is not None and hg == 0 and kt == 3:
                        finish_qt(*pending)
                        pending = None
                    dlt = k0 - q0      # >0 only on diagonal k-tiles
                    stp = ps_sc.tile([P, 2, NT], F32, tag="sc", name="sc")
                    es = esp.tile([P, 2, NT], BF16, tag="es", name="es")
                    for u in range(2):
                        hp = slice(64 * u, 64 * u + 64)
                        if dlt > 0:
                            nc.tensor.matmul(stp[:, u, dlt:],
                                             qkT[2 + hg][hp, k0:k0 + KT],
                                             qkT[hg][hp, q0 + dlt:q0 + NT],
                                             start=True, stop=True)
                        else:
                            nc.tensor.matmul(stp[:, u, :],
                                             qkT[2 + hg][hp, k0:k0 + KT],
                                             qkT[hg][hp, q0:q0 + NT],
                                             start=True, stop=True)
                    if dlt > 0:
                        nc.vector.memset(es[:, :, 0:dlt], 0.0)
                        nc.scalar.activation(es[:, :, dlt:],
                                             stp[:, :, dlt:], AF.Exp,
                                             scale=1.0 / math.sqrt(D))
                    else:
                        nc.scalar.activation(es, stp, AF.Exp,
                                             scale=1.0 / math.sqrt(D))
                    if kt >= 4 * qt:   # diagonal triangle
                        for u in range(2):
                            nc.vector.tensor_mul(es[:, u, dlt:dlt + KT],
                                                 es[:, u, dlt:dlt + KT],
                                                 tri)
                    for u in range(2):
                        hl = 2 * hg + u
                        nc.tensor.matmul(
                            cxs2[u],
                            vaug[kt][:, hl * (D + 1):(hl + 1) * (D + 1)],
                            es[:, u, :],
                            start=(kt == 0), stop=(kt == 4 * qt + 3))
                # drain the pass: ctx out of PSUM fast (frees the banks for
                # the next pass) and collect the denominator rows on
                # partitions {0,32,64,96} for the strided Newton reciprocal
                for u in range(2):
                    hl = 2 * hg + u
                    cr = recp.tile([D + 1, NT], BF16, tag=f"cr{hl}",
                                   name=f"cr{hl}")
                    nc.vector.tensor_copy(cr, cxs2[u])
                    ctxR[hl] = cr
                    nc.vector.tensor_copy(den4[32 * hl:32 * hl + 1, :],
                                          cxs2[u][D:D + 1, :])
            pending = (qt, q0, den4, ctxR)
        finish_qt(*pending)

    # ---- 4. receiver: assemble ctx (chunk 3 in separate tiles), output
    # projection pass A (chunks 0-2) + pass B + residual ----
    with tc.tile_pool(name="ps_o", bufs=4, space="PSUM") as ps_o, \
         tc.tile_pool(name="yp", bufs=2) as yp:
        for k in range(QT):
            for i in range(FT):
                dst = (ctxA[i][:, 2 * CH * k:2 * CH * (k + 1)]
                       if k < 3 else ctxB[i][:, :])
                nc.sync.dma_start(
                    out=dst.rearrange("p (b q) -> p b q", b=2),
                    in_=a2a_out[k][i // 2::4,
                                   P * (i % 2):P * (i % 2) + P, :].rearrange(
                        "b r q -> r b q"))

        def opro(src, lo, hi, tag):
            for mt in range(FT):
                ps = ps_o.tile([P, hi - lo], F32, tag=f"o{tag}",
                               name=f"o{tag}")
                for k in range(FT):
                    nc.tensor.matmul(ps, wo[k][:, mt * P:(mt + 1) * P],
                                     src[k],
                                     start=(k == 0), stop=(k == FT - 1))
                ysb = yp.tile([P, hi - lo], F32, tag=f"y{tag}",
                              name=f"y{tag}")
                nc.vector.tensor_add(ysb, ps, xres[mt][:, lo:hi])
                nc.sync.dma_start(out=y_d[mt * P:(mt + 1) * P, lo:hi],
                                  in_=ysb)

        opro(ctxA, 0, 6 * CH, "a")
        opro(ctxB, 6 * CH, 8 * CH, "b")


def _drpack(a):
    """[1024, N] -> [512, 2N]: row j*128+p holds source rows (256j+p,
    256j+128+p) interleaved as two N-blocks (DoubleRow layout)."""
    n = a.shape[1]
    return np.ascontiguousarray(
        a.reshape(4, 2, 128, n).transpose(0, 2, 1, 3).reshape(512, 2 * n))


def _prep_inputs(x, ln_g, ln_b, wqkv, bqkv, wo, bo):
    """Host-side sharding / folding. Returns per-core input dicts."""
    f32 = np.float32
    bf16 = ml_dtypes.bfloat16
    fp8 = ml_dtypes.float8_e4m3
    ws = np.float32(WS)
    x = np.asarray(x, f32)
    wg = (np.asarray(wqkv, f32) * np.asarray(ln_g, f32)[:, None])
    tri = (np.arange(128)[None, :] >= np.arange(128)[:, None]).astype(bf16)
    wo_bf = np.asarray(wo, f32).astype(bf16)
    bo_f = np.asarray(bo, f32)
    lnb = np.asarray(ln_b, f32)
    bq = np.asarray(bqkv, f32)
    ws = np.float32(WS)

    xT = [np.ascontiguousarray(x[b].T) for b in range(B)]
    xdr = [_drpack(t).astype(fp8) for t in xT]

    maps = []
    for c in range(N_CORES):
        b, s = divmod(c, 4)
        qs = slice(DL * s, DL * s + DL)
        ks = slice(DIM + DL * s, DIM + DL * s + DL)
        vs = slice(2 * DIM + DL * s, 2 * DIM + DL * s + DL)
        wqk_8 = (np.concatenate([wg[:, qs], wg[:, ks]], axis=1) * ws).astype(fp8)
        wv_8 = (wg[:, vs] * ws).astype(fp8)
        wqk_f = wqk_8.astype(f32)                # quantized weights (x WS)
        wv_f = wv_8.astype(f32)
        cqk = wqk_f.sum(0)                       # [512], scaled by WS
        b2qk = (np.concatenate([bq[qs], bq[ks]]) + (wqk_f.T @ lnb) / ws) * ws
        cv = wv_f.sum(0)                         # [256], scaled by WS
        b2v = (bq[vs] + (wv_f.T @ lnb) / ws) * ws
        # output cols: c2 = k*128 + b2*64 + q  <->  s = k*512 + 64*c + q
        xres = np.concatenate(
            [xT[b][:, k * 512 + 64 * c:k * 512 + 64 * c + CH]
             for k in range(QT) for b in range(B)],
            axis=1) + bo_f[:, None]
        maps.append({
            "xdr": xdr[b],
            "xres": np.ascontiguousarray(xres),
            "wqk": _drpack(wqk_8.astype(f32)).astype(fp8),
            "wv": _drpack(wv_8.astype(f32)).astype(fp8),
            "wo": wo_bf,
            "qcor": np.stack([-cqk, b2qk]).astype(bf16),
            "vcor": np.stack([-cv, b2v]).astype(bf16),
            "tri": tri,
        })
    return maps


def kernel(**inputs):
    if "nc" not in _CACHE:
        _CACHE["nc"] = _build()
    nc = _CACHE["nc"]
    maps = _prep_inputs(**inputs)
    res = run_bass_kernel_spmd(nc, maps, list(range(N_CORES)))
    out = np.empty((B, S, DIM), np.float32)
    for c in range(N_CORES):
        y = res.results[c]["y"]            # [DIM, 512], cols (k, b2, q)
        for k in range(QT):
            for b2 in range(B):
                out[b2, k * 512 + 64 * c:k * 512 + 64 * c + CH, :] = \
                    y[:, 2 * CH * k + CH * b2:2 * CH * k + CH * (b2 + 1)].T
    return out


# revision 97
# speedup vs baseline: 1.1390x; 1.1390x over previous
"""Trainium2 Bass kernel for pre-norm causal attention block.

Module: out = x + Wo(attn(LN(x))) with fused QKV, 16 heads, causal mask.
Shapes (hardcoded): x [2, 2048, 1024], wqkv [1024, 3072], wo [1024, 1024].

Sharding (8 cores, one program SPMD):
  core c = 4*b + s handles batch b, global heads [4s, 4s+4). Attention
  context is normalized on the sender, then exchanged with 4 chunked
  AllToAlls (one per 512-wide q block, each shipping a 64-col slice per
  destination) so the exchange overlaps attention compute. Core c's output
  rows are seq positions {qt*512 + 64*c + q : qt in 0..3, q in 0..63} of
  both batches.

Dataflow per core (feature-on-partitions, i.e. transposed):
  1. x, wqkv, wv arrive as fp8-e4m3 in DoubleRow-packed layout (weights
     pre-scaled x64 on the host to dodge e4m3 subnormals). LN stats via
     DoubleRow ones-matmuls on PE; rsqrt via fast-inverse-sqrt + Newton on
     DVE. LN is folded into the projections via two extra bf16 contraction
     rows: psum = 64*Wg^T x^T - 64*C*mu + 64*b2*(1/r); qkv^T = (r/64)*psum,
     so the per-tile epilogue is one DVE scalar_tensor_tensor with a
     0-stride-DMA broadcast of r.
  2. QK projection (fp8 DoubleRow, K=256/step) -> qk^T bf16 [512, 2048];
     V projection with swapped operands -> V row layout [2048, 256] (+ a
     ones column per head for softmax denominators), epilogue = one
     tensor_scalar by r/64 (per-partition).
  3. Per head: scores^T blocks [128 k, q] (bf16, K=64) for the visible
     (causal) q-range only, exp on ACT (scale=1/8 folded in), 128x128
     triangular mask on the diagonal block, ctx^T = V_aug^T @ expS (bf16)
     accumulated in PSUM [65, 512] (row 64 = softmax denominator).
  4. Per qt: ctx copied out of PSUM fast (frees banks), denominator
     reciprocals via Newton on a partition-{0,32,64,96} strided tile,
     broadcast via one selector matmul per head group, normalize into ctxT
     (bf16), ship per-dest 64-col slices with merged DMAs, then one
     AllToAll per qt so chunks 0-2 overlap attention compute (a tiny
     warmup AllToAll at kernel start absorbs the CC ring-setup cost).
  5. Receiver: assemble ctx_all [128, 512] (cols ordered chunk-major),
     output projection split into pass A (chunks 0-2, overlaps the last
     AllToAll) and pass B + residual (bias pre-folded into xres on host).
"""

import sys

for _p in ("/opt/trn_rl_repo",):
    if _p not in sys.path:
        sys.path.insert(0, _p)

import math

import ml_dtypes
import numpy as np

import concourse.bass as bass
import concourse.mybir as mybir
import concourse.tile as tile
from concourse import bacc
from concourse.bass_utils import run_bass_kernel_spmd

F32 = mybir.dt.float32
BF16 = mybir.dt.bfloat16
FP8 = mybir.dt.float8e4
I32 = mybir.dt.int32
AF = mybir.ActivationFunctionType
ALU = mybir.AluOpType
DR = mybir.MatmulPerfMode.DoubleRow
WS = 64.0                # fp8 weight scale (avoids e4m3 subnormals)

N_CORES = 8
B, S, H, D = 2, 2048, 16, 64
DIM = H * D              # 1024
HL = 4                   # heads per core
DL = HL * D              # 256 local head features
EPS = 1e-6
KT = 128                 # k-tile (partition) width
NT = 512                 # matmul free-dim tile
FT = DIM // KT           # 8 feature tiles
ST = S // KT             # 16 seq tiles of 128
QT = S // NT             # 4 q-tiles of 512
CH = 64                  # per-destination column slice per chunk

_CACHE = {}


def _build():
    nc = bacc.Bacc("TRN2", target_bir_lowering=False, debug=False,
                   num_devices=N_CORES)

    # ---- I/O ----
    # DoubleRow-packed fp8: row j*128+p holds features (256j+p, 256j+128+p)
    # interleaved as [2, N] blocks
    xdr_d = nc.dram_tensor("xdr", [DIM // 2, 2 * S], FP8, kind="ExternalInput")
    xres_d = nc.dram_tensor("xres", [DIM, NT], F32, kind="ExternalInput")
    wqk_d = nc.dram_tensor("wqk", [DIM // 2, 4 * DL], FP8,
                           kind="ExternalInput")
    wv_d = nc.dram_tensor("wv", [DIM // 2, 2 * DL], FP8, kind="ExternalInput")
    wo_d = nc.dram_tensor("wo", [DIM, DIM], BF16, kind="ExternalInput")
    qcor_d = nc.dram_tensor("qcor", [2, 2 * DL], BF16, kind="ExternalInput")
    vcor_d = nc.dram_tensor("vcor", [2, DL], BF16, kind="ExternalInput")
    tri_d = nc.dram_tensor("tri", [128, 128], BF16, kind="ExternalInput")
    y_d = nc.dram_tensor("y", [DIM, NT], F32, kind="ExternalOutput")

    # ---- DRAM scratch ----
    stats_dram = nc.dram_tensor("stats_dram", [2, S], F32)
    rec_dram = nc.dram_tensor("rec_dram", [QT, HL, NT], F32)
    recip_dram = nc.dram_tensor("recip_dram", [QT, HL, NT], F32)
    r_dram = nc.dram_tensor("r_dram", [S], F32)
    a2a_in = [nc.dram_tensor(f"a2a_in{k}", [N_CORES, 2 * KT, CH], BF16)
              for k in range(QT)]
    a2a_out = [nc.dram_tensor(f"a2a_out{k}", [N_CORES, 2 * KT, CH], BF16)
               for k in range(QT)]
    warm_in = nc.dram_tensor("warm_in", [N_CORES, 1, CH], BF16)
    warm_out = nc.dram_tensor("warm_out", [N_CORES, 1, CH], BF16)

    with tile.TileContext(nc) as tc:
        import contextlib
        with contextlib.ExitStack() as ctx:
            _build_body(ctx, tc, nc, locals())
    nc.compile()
    return nc


def _build_body(ctx, tc, nc, t):
    xdr_d, xres_d, wqk_d, wv_d, wo_d = (t["xdr_d"], t["xres_d"], t["wqk_d"],
                                        t["wv_d"], t["wo_d"])
    qcor_d, vcor_d, tri_d, y_d = t["qcor_d"], t["vcor_d"], t["tri_d"], t["y_d"]
    stats_dram, a2a_in, a2a_out = t["stats_dram"], t["a2a_in"], t["a2a_out"]
    rec_dram = t["rec_dram"]
    recip_dram = t["recip_dram"]
    r_dram = t["r_dram"]

    P = 128
    sing = ctx.enter_context(tc.tile_pool(name="sing", bufs=1))
    # persistent SBUF tiles
    JT = FT // 2             # 4 DoubleRow feature steps of 256
    xdr = [sing.tile([P, 2, S], FP8, tag=f"xdr{j}", name=f"xdr{j}")
           for j in range(JT)]
    xres = [sing.tile([P, NT], F32, tag=f"xres{i}", name=f"xres{i}") for i in range(FT)]
    wqk = [sing.tile([P, 2, 2 * DL], FP8, tag=f"wqk{j}", name=f"wqk{j}")
           for j in range(JT)]
    wv = [sing.tile([P, 2, DL], FP8, tag=f"wv{j}", name=f"wv{j}")
          for j in range(JT)]
    wo = [sing.tile([P, DIM], BF16, tag=f"wo{i}", name=f"wo{i}")
          for i in range(FT)]
    qkT = [sing.tile([P, S], BF16, tag=f"qkT{i}", name=f"qkT{i}") for i in range(4)]
    vaug = [sing.tile([P, HL * (D + 1)], BF16, tag=f"vaug{i}", name=f"vaug{i}")
            for i in range(ST)]
    ctxT = [sing.tile([P, S], BF16, tag=f"ctxT{i}", name=f"ctxT{i}") for i in range(2)]
    # chunks 0-2 and chunk 3 in physically separate tiles so the pass-A
    # output projection has no (tile-granular) dependency on the last chunk
    ctxA = [sing.tile([P, 6 * CH], BF16, tag=f"cta{i}", name=f"cta{i}")
            for i in range(FT)]
    ctxB = [sing.tile([P, 2 * CH], BF16, tag=f"ctb{i}", name=f"ctb{i}")
            for i in range(FT)]
    rB = sing.tile([P, S], F32, tag="rB")
    mrow = sing.tile([2, S], BF16, tag="mrow")
    qcor_c = sing.tile([2, 2 * DL], BF16, tag="qcor")
    vcor_c = sing.tile([2, DL], BF16, tag="vcor")
    tri = sing.tile([P, P], BF16, tag="tri")
    ones = sing.tile([P, 1], BF16, tag="ones")

    rcP = sing.tile([P, ST], F32, tag="rcP")

    # input DMAs - x first (stats critical path), weights next, rest last
    for j in range(JT):
        nc.sync.dma_start(
            out=xdr[j],
            in_=xdr_d[j * P:(j + 1) * P, :].rearrange("p (t s) -> p t s", t=2))
    for j in range(JT):
        nc.sync.dma_start(
            out=wqk[j],
            in_=wqk_d[j * P:(j + 1) * P, :].rearrange("p (t s) -> p t s", t=2))
    for j in range(JT):
        nc.sync.dma_start(
            out=wv[j],
            in_=wv_d[j * P:(j + 1) * P, :].rearrange("p (t s) -> p t s", t=2))
    nc.sync.dma_start(out=qcor_c, in_=qcor_d[:])
    nc.sync.dma_start(out=vcor_c, in_=vcor_d[:])
    nc.sync.dma_start(out=tri, in_=tri_d[:])
    for i in range(FT):
        nc.sync.dma_start(out=wo[i], in_=wo_d[i * P:(i + 1) * P, :])
        nc.sync.dma_start(out=xres[i], in_=xres_d[i * P:(i + 1) * P, :])
    nc.vector.memset(ones, 1.0)

    # tiny warmup AllToAll: pays the CC ring-setup cost (~45us) during the
    # stats/QK phase so the real chunked exchanges run at steady state
    nc.gpsimd.collective_compute(
        "AllToAll", ALU.bypass,
        replica_groups=[list(range(N_CORES))],
        ins=[t["warm_in"][:].opt()], outs=[t["warm_out"][:].opt()],
        unique_tensors="Yes")

    # ---- 1. LN stats: column sums of x and x^2 via DoubleRow ones-matmul ----
    ones_f8 = sing.tile([P, 2, 16], FP8, tag="ones_f8")
    nc.vector.memset(ones_f8, 1.0)
    stats_sa = sing.tile([1, S], F32, tag="stats_sa")
    stats_sq = sing.tile([1, S], F32, tag="stats_sq")
    # stats use only 2 PSUM banks (sum/sq per nt sequentially) so the QK
    # pool below coexists and the PE never stalls on a pool release
    with tc.tile_pool(name="ps_st", bufs=4, space="PSUM") as ps_st, \
         tc.tile_pool(name="sqp", bufs=2) as sqp:
        sps = [ps_st.tile([1, NT], F32, tag="sum", name=f"sum{nt}")
               for nt in range(QT)]
        qps = [ps_st.tile([1, NT], F32, tag="sq", name=f"sqp{nt}")
               for nt in range(QT)]
        for j in range(JT):
            sq = sqp.tile([P, 2, S], FP8, tag="sq", name="sq")
            # squares on ACT (idle at startup; Square shares the exp table)
            # so the DVE queue stays clear for the stats->mrow chain
            nc.scalar.activation(sq, xdr[j], AF.Square)
            for nt in range(QT):
                sl = slice(nt * NT, (nt + 1) * NT)
                nc.tensor.matmul(sps[nt], ones_f8[:, :, 0:1],
                                 xdr[j][:, :, sl],
                                 start=(j == 0), stop=(j == JT - 1),
                                 perf_mode=DR)
                nc.tensor.matmul(qps[nt], ones_f8[:, :, 0:1], sq[:, :, sl],
                                 start=(j == 0), stop=(j == JT - 1),
                                 perf_mode=DR)
        for nt in range(QT):
            sl = slice(nt * NT, (nt + 1) * NT)
            nc.vector.tensor_copy(stats_sa[:, sl], sps[nt])
            nc.vector.tensor_copy(stats_sq[:, sl], qps[nt])
    nc.sync.dma_start(out=stats_dram[0:1], in_=stats_sa[:])
    nc.sync.dma_start(out=stats_dram[1:2], in_=stats_sq[:])
    # contiguous [16,128] reads, math at 16 partitions, then flatten
    # (SBUF->SBUF DMA) for the correction rows / broadcasts
    idn = sing.tile([P, P], F32, tag="idn")
    from concourse.masks import make_identity
    make_identity(nc, idn)
    sPT = sing.tile([16, P], F32, tag="sPT")
    qPT = sing.tile([16, P], F32, tag="qPT")
    nc.sync.dma_start(out=sPT, in_=stats_dram[0].rearrange("(j p) -> j p", j=16))
    nc.sync.dma_start(out=qPT, in_=stats_dram[1].rearrange("(j p) -> j p", j=16))
    muT = sing.tile([16, P], F32, tag="muT")
    nc.vector.tensor_scalar(muT, sPT, 1.0 / DIM, None, op0=ALU.mult)
    nc.vector.tensor_scalar(qPT, qPT, 1.0 / DIM, None, op0=ALU.mult)
    t0 = sing.tile([16, P], F32, tag="t0")
    nc.vector.tensor_mul(t0, muT, muT)
    nc.vector.tensor_sub(t0, qPT, t0)
    nc.vector.tensor_scalar(t0, t0, EPS, None, op0=ALU.add)
    # rsqrt via fast-inverse-square-root seed + 3 Newton steps
    rT = sing.tile([16, P], F32, tag="rT")
    t1s = sing.tile([16, P], F32, tag="t1s")
    nc.vector.tensor_scalar(rT[:].bitcast(I32), t0[:].bitcast(I32), 1, None,
                            op0=ALU.logical_shift_right)
    nc.vector.tensor_scalar(rT[:].bitcast(I32), rT[:].bitcast(I32), -1, None,
                            op0=ALU.bitwise_xor)
    nc.vector.tensor_scalar(rT[:].bitcast(I32), rT[:].bitcast(I32),
                            0x5F3759E0, None, op0=ALU.add)
    for _ in range(2):
        nc.vector.tensor_mul(t1s, rT, rT)
        nc.vector.tensor_mul(t1s, t1s, t0)
        nc.vector.tensor_scalar(t1s, t1s, -0.5, 1.5, op0=ALU.mult,
                                op1=ALU.add)
        nc.vector.tensor_mul(rT, rT, t1s)
    # invr = sqrt(var+eps) = t0 * r ; bf16 copies of mu and invr for the
    # correction rows
    invrT = sing.tile([16, P], F32, tag="invrT")
    nc.vector.tensor_mul(invrT, t0, rT)
    muB = sing.tile([16, P], BF16, tag="muB")
    invrB = sing.tile([16, P], BF16, tag="invrB")
    nc.vector.tensor_copy(muB, muT)
    nc.vector.tensor_copy(invrB, invrT)
    # mrow rows: [mu; invr] in seq order (partition-major flatten)
    nc.sync.dma_start(out=mrow[0:1, :], in_=muB[:])
    nc.sync.dma_start(out=mrow[1:2, :], in_=invrB[:])
    # r to DRAM, then 0-stride DMA broadcast to all 128 partitions
    nc.sync.dma_start(out=r_dram[:], in_=rT[:])
    nc.sync.dma_start(out=rB[:], in_=r_dram[:].partition_broadcast(P))

    # ---- 2. QK projection with LN correction rows ----
    with tc.tile_pool(name="ps_qk", bufs=8, space="PSUM") as ps_qk:
        for mt in range(4):          # qkT M-tiles (Q01 Q23 K01 K23)
            pss = []
            for nt in range(QT):
                sl = slice(nt * NT, (nt + 1) * NT)
                ps = ps_qk.tile([P, NT], F32, tag="qk", name=f"qk{nt}")
                for j in range(JT):
                    nc.tensor.matmul(
                        ps, wqk[j][:, :, mt * P:(mt + 1) * P],
                        xdr[j][:, :, sl],
                        start=(j == 0), stop=False, perf_mode=DR)
                pss.append(ps)
            for nt in range(QT):
                sl = slice(nt * NT, (nt + 1) * NT)
                nc.tensor.matmul(pss[nt], qcor_c[:, mt * P:(mt + 1) * P],
                                 mrow[:, sl], start=False, stop=True)
                nc.vector.scalar_tensor_tensor(
                    qkT[mt][:, sl], pss[nt], 1.0 / WS, rB[:, sl],
                    op0=ALU.mult, op1=ALU.mult)
    # column layout of r for the V epilogue via PE transpose (emitted after
    # the QK matmuls so the rT dependency never stalls the PE)
    with tc.tile_pool(name="ps_tp", bufs=1, space="PSUM") as ps_tp:
        tp = ps_tp.tile([P, 16], F32, tag="tp", name="tp")
        nc.tensor.transpose(tp, rT[:], idn[0:16, 0:16])
        # fold the 1/WS fp8 weight descale into the per-partition r column
        nc.vector.tensor_scalar(rcP, tp, 1.0 / WS, None, op0=ALU.mult)

    # selectors for the recip-broadcast matmuls (head hl's recip row lives
    # on partition 32*hl): selg[hg] maps rows {64hg, 64hg+32} to output
    # partition halves; zero rows null out the never-written garbage lanes
    selg = [sing.tile([97, P], BF16, tag=f"selg{hg}", name=f"selg{hg}")
            for hg in range(2)]
    for hg in range(2):
        nc.vector.memset(selg[hg], 0.0)
        nc.vector.memset(selg[hg][64 * hg:64 * hg + 1, 0:64], 1.0)
        nc.vector.memset(selg[hg][64 * hg + 32:64 * hg + 33, 64:128], 1.0)

    # ---- 3. attention: q-outer, 4 heads interleaved, V proj woven in ----
    with tc.tile_pool(name="ps_sc", bufs=2, space="PSUM") as ps_sc, \
         tc.tile_pool(name="ps_cx", bufs=1, space="PSUM") as ps_cx, \
         tc.tile_pool(name="ps_v", bufs=1, space="PSUM") as ps_v, \
         tc.tile_pool(name="ps_rb", bufs=1, space="PSUM") as ps_rb, \
         tc.tile_pool(name="esp", bufs=4) as esp, \
         tc.tile_pool(name="recp", bufs=2) as recp, \
         tc.tile_pool(name="rbp", bufs=4) as rbp:
        v_done = set()

        def weave_v(st):
            if st in v_done:
                return
            v_done.add(st)
            ps = ps_v.tile([P, DL], F32, tag="v", name="v")
            for j in range(JT):
                nc.tensor.matmul(
                    ps, xdr[j][:, :, st * P:(st + 1) * P], wv[j],
                    start=(j == 0), stop=False, perf_mode=DR)
            nc.tensor.matmul(ps, mrow[:, st * P:(st + 1) * P], vcor_c[:],
                             start=False, stop=True)
            vw = vaug[st][:].rearrange("p (h e) -> p h e", h=HL)
            nc.vector.tensor_scalar(
                vw[:, :, 0:D], ps.rearrange("p (h d) -> p h d", h=HL),
                rcP[:, st:st + 1], None, op0=ALU.mult)
            nc.vector.memset(vw[:, :, D:D + 1], 1.0)

        def finish_qt(qt, q0, den4, ctxR):
            """Newton reciprocal + normalize + ship for one q-block; emitted
            a few k-tiles into the NEXT q-block so the PE never head-of-line
            blocks on this (DVE-serial) chain at the boundary."""
            y0 = recp.tile([97, NT], F32, tag="y0", name="y0")
            a4 = recp.tile([97, NT], F32, tag="a4", name="a4")
            y0b = recp.tile([97, NT], BF16, tag="y0b", name="y0b")
            nc.vector.tensor_scalar(y0[:].bitcast(I32), den4[:].bitcast(I32),
                                    -1, None, op0=ALU.bitwise_xor)
            nc.vector.tensor_scalar(y0[:].bitcast(I32), y0[:].bitcast(I32),
                                    0x7EF311C4, None, op0=ALU.add)
            for _ in range(2):
                nc.vector.tensor_mul(a4, den4, y0)
                nc.vector.tensor_scalar(a4, a4, -1.0, 2.0,
                                        op0=ALU.mult, op1=ALU.add)
                nc.vector.tensor_mul(y0, y0, a4)
            nc.vector.tensor_copy(y0b, y0)
            # per head group: broadcast both recip rows via one selector
            # matmul, then normalize while writing into ctxT (bf16)
            for hg in range(2):
                rbps = ps_rb.tile([P, NT], F32, tag="rb", name="rb")
                nc.tensor.matmul(rbps, selg[hg][:], y0b[:],
                                 start=True, stop=True)
                for u in range(2):
                    hl = 2 * hg + u
                    nc.vector.tensor_mul(
                        ctxT[hg][64 * u:64 * u + 64, q0:q0 + NT],
                        ctxR[hl][0:D, :], rbps[64 * u:64 * u + 64, :])
            # ship this qt's 64-col window to each destination core (one
            # merged DMA per ctxT tile)
            for g in range(2):
                nc.sync.dma_start(
                    out=a2a_in[qt][:, P * g:P * (g + 1), :].rearrange(
                        "j r q -> r j q"),
                    in_=ctxT[g][:, q0:q0 + NT].rearrange(
                        "p (j q) -> p j q", j=N_CORES))
            nc.gpsimd.collective_compute(
                "AllToAll", ALU.bypass,
                replica_groups=[list(range(N_CORES))],
                ins=[a2a_in[qt][:].opt()], outs=[a2a_out[qt][:].opt()],
                unique_tensors="Yes")

        pending = None
        for qt in range(QT):
            q0 = qt * NT
            den4 = recp.tile([97, NT], F32, tag="den4", name="den4")
            nc.vector.memset(den4, 1.0)
            ctxR = [None] * HL
            # two head-group passes: only 2 ctx accumulators live at once
            # (2 PSUM banks) and each exp covers both heads of the group in
            # one [128, 2, 512] instruction (half the ACT dispatch overhead)
            for hg in range(2):
                cxs2 = [ps_cx.tile([D + 1, NT], F32, tag=f"cx{u}",
                                   name=f"cx{u}") for u in range(2)]
                for kt in range(4 * qt + 4):
                    k0 = kt * KT
                    weave_v(kt)
                    if pending is not None and hg == 0 and kt == 3:
                        finish_qt(*pending)
                        pending = None
                    dlt = k0 - q0      # >0 only on diagonal k-tiles
                    stp = ps_sc.tile([P, 2, NT], F32, tag="sc", name="sc")
                    es = esp.tile([P, 2, NT], BF16, tag="es", name="es")
                    for u in range(2):
                        hp = slice(64 * u, 64 * u + 64)
                        if dlt > 0:
                            nc.tensor.matmul(stp[:, u, dlt:],
                                             qkT[2 + hg][hp, k0:k0 + KT],
                                             qkT[hg][hp, q0 + dlt:q0 + NT],
                                             start=True, stop=True)
                        else:
                            nc.tensor.matmul(stp[:, u, :],
                                             qkT[2 + hg][hp, k0:k0 + KT],
                                             qkT[hg][hp, q0:q0 + NT],
                                             start=True, stop=True)
                    if dlt > 0:
                        nc.vector.memset(es[:, :, 0:dlt], 0.0)
                        nc.scalar.activation(es[:, :, dlt:],
                                             stp[:, :, dlt:], AF.Exp,
                                             scale=1.0 / math.sqrt(D))
                    else:
                        nc.scalar.activation(es, stp, AF.Exp,
                                             scale=1.0 / math.sqrt(D))
                    if kt >= 4 * qt:   # diagonal triangle
                        for u in range(2):
                            nc.vector.tensor_mul(es[:, u, dlt:dlt + KT],
                                                 es[:, u, dlt:dlt + KT],
                                                 tri)
                    for u in range(2):
                        hl = 2 * hg + u
                        nc.tensor.matmul(
                            cxs2[u],
                            vaug[kt][:, hl * (D + 1):(hl + 1) * (D + 1)],
                            es[:, u, :],
                            start=(kt == 0), stop=(kt == 4 * qt + 3))
                # drain the pass: ctx out of PSUM fast (frees the banks for
                # the next pass) and collect the denominator rows on
                # partitions {0,32,64,96} for the strided Newton reciprocal
                for u in range(2):
                    hl = 2 * hg + u
                    cr = recp.tile([D + 1, NT], BF16, tag=f"cr{hl}",
                                   name=f"cr{hl}")
                    nc.vector.tensor_copy(cr, cxs2[u])
                    ctxR[hl] = cr
                    nc.vector.tensor_copy(den4[32 * hl:32 * hl + 1, :],
                                          cxs2[u][D:D + 1, :])
            pending = (qt, q0, den4, ctxR)
        finish_qt(*pending)

    # ---- 4. receiver: assemble ctx (chunk 3 in separate tiles), output
    # projection pass A (chunks 0-2) + pass B + residual ----
    with tc.tile_pool(name="ps_o", bufs=4, space="PSUM") as ps_o, \
         tc.tile_pool(name="yp", bufs=2) as yp:
        for k in range(QT):
            for i in range(FT):
                for b2 in range(2):
                    dst = (ctxA[i][:, 2 * CH * k + CH * b2:
                                   2 * CH * k + CH * (b2 + 1)]
                           if k < 3 else
                           ctxB[i][:, CH * b2:CH * (b2 + 1)])
                    nc.sync.dma_start(
                        out=dst,
                        in_=a2a_out[k][i // 2 + 4 * b2,
                                       P * (i % 2):P * (i % 2) + P, :])

        def opro(src, lo, hi, tag):
            for mt in range(FT):
                ps = ps_o.tile([P, hi - lo], F32, tag=f"o{tag}",
                               name=f"o{tag}")
                for k in range(FT):
                    nc.tensor.matmul(ps, wo[k][:, mt * P:(mt + 1) * P],
                                     src[k],
                                     start=(k == 0), stop=(k == FT - 1))
                ysb = yp.tile([P, hi - lo], F32, tag=f"y{tag}",
                              name=f"y{tag}")
                nc.vector.tensor_add(ysb, ps, xres[mt][:, lo:hi])
                nc.sync.dma_start(out=y_d[mt * P:(mt + 1) * P, lo:hi],
                                  in_=ysb)

        opro(ctxA, 0, 6 * CH, "a")
        opro(ctxB, 6 * CH, 8 * CH, "b")


def _drpack(a):
    """[1024, N] -> [512, 2N]: row j*128+p holds source rows (256j+p,
    256j+128+p) interleaved as two N-blocks (DoubleRow layout)."""
    n = a.shape[1]
    return np.ascontiguousarray(
        a.reshape(4, 2, 128, n).transpose(0, 2, 1, 3).reshape(512, 2 * n))


def _prep_inputs(x, ln_g, ln_b, wqkv, bqkv, wo, bo):
    """Host-side sharding / folding. Returns per-core input dicts."""
    f32 = np.float32
    bf16 = ml_dtypes.bfloat16
    fp8 = ml_dtypes.float8_e4m3
    ws = np.float32(WS)
    x = np.asarray(x, f32)
    wg = (np.asarray(wqkv, f32) * np.asarray(ln_g, f32)[:, None])
    tri = (np.arange(128)[None, :] >= np.arange(128)[:, None]).astype(bf16)
    wo_bf = np.asarray(wo, f32).astype(bf16)
    bo_f = np.asarray(bo, f32)
    lnb = np.asarray(ln_b, f32)
    bq = np.asarray(bqkv, f32)
    ws = np.float32(WS)

    xT = [np.ascontiguousarray(x[b].T) for b in range(B)]
    xdr = [_drpack(t).astype(fp8) for t in xT]

    maps = []
    for c in range(N_CORES):
        b, s = divmod(c, 4)
        qs = slice(DL * s, DL * s + DL)
        ks = slice(DIM + DL * s, DIM + DL * s + DL)
        vs = slice(2 * DIM + DL * s, 2 * DIM + DL * s + DL)
        wqk_8 = (np.concatenate([wg[:, qs], wg[:, ks]], axis=1) * ws).astype(fp8)
        wv_8 = (wg[:, vs] * ws).astype(fp8)
        wqk_f = wqk_8.astype(f32)                # quantized weights (x WS)
        wv_f = wv_8.astype(f32)
        cqk = wqk_f.sum(0)                       # [512], scaled by WS
        b2qk = (np.concatenate([bq[qs], bq[ks]]) + (wqk_f.T @ lnb) / ws) * ws
        cv = wv_f.sum(0)                         # [256], scaled by WS
        b2v = (bq[vs] + (wv_f.T @ lnb) / ws) * ws
        # output cols: c2 = k*128 + b2*64 + q  <->  s = k*512 + 64*c + q
        xres = np.concatenate(
            [xT[b][:, k * 512 + 64 * c:k * 512 + 64 * c + CH]
             for k in range(QT) for b in range(B)],
            axis=1) + bo_f[:, None]
        maps.append({
            "xdr": xdr[b],
            "xres": np.ascontiguousarray(xres),
            "wqk": _drpack(wqk_8.astype(f32)).astype(fp8),
            "wv": _drpack(wv_8.astype(f32)).astype(fp8),
            "wo": wo_bf,
            "qcor": np.stack([-cqk, b2qk]).astype(bf16),
            "vcor": np.stack([-cv, b2v]).astype(bf16),
            "tri": tri,
        })
    return maps


def kernel(**inputs):
    if "nc" not in _CACHE:
        _CACHE["nc"] = _build()
    nc = _CACHE["nc"]
    maps = _prep_inputs(**inputs)
    res = run_bass_kernel_spmd(nc, maps, list(range(N_CORES)))
    out = np.empty((B, S, DIM), np.float32)
    for c in range(N_CORES):
        y = res.results[c]["y"]            # [DIM, 512], cols (k, b2, q)
        for k in range(QT):
            for b2 in range(B):
                out[b2, k * 512 + 64 * c:k * 512 + 64 * c + CH, :] = \
                    y[:, 2 * CH * k + CH * b2:2 * CH * k + CH * (b2 + 1)].T
    return out
